# revision 1
# baseline (speedup 1.0000x reference)
"""NF4-style 4-bit quantized linear: out = x @ dequant(w).T on 8 TRN2 NeuronCores.

Column-parallel sharding: core c owns output features [c*512, (c+1)*512) and the
corresponding contiguous slices of the packed weight + quant state arrays. x is
replicated. Each core:
  1. dequantizes its 512x4096 weight slice on-chip (DVE) into fp16,
  2. round-trips it through DRAM with an xbar transpose DMA to get wT
     [k-partition, outf] layout,
  3. streams x through xbar transpose DMAs ([token, k] -> [k, token]) and runs
     the fp16 matmul on the PE array, accumulating in PSUM over 32 k-tiles.
Host gathers the per-core [8192, 512] outputs with a concat along axis 1.
"""
import numpy as np

import concourse.bass as bass
import concourse.mybir as mybir
import concourse.tile as tile
from concourse import bacc
from concourse.tile_rust import add_dep_helper as tile_rust_add_dep
from concourse.bass_utils import run_bass_kernel_spmd

F16 = mybir.dt.float16
F32 = mybir.dt.float32
I32 = mybir.dt.int32
Alu = mybir.AluOpType

P = 128
TOKENS = 8192
IN_F = 4096
OUT_F = 4096
N_CORES = 8
O_C = OUT_F // N_CORES          # 512 out features per core
KT = IN_F // P                  # 32 k-tiles
BPR = IN_F // 2                 # 2048 packed bytes per weight row
NB_O = O_C // P                 # 4 o-tiles of 128 rows
TB = 512                        # token block
BC = 2                          # byte-column chunks per o-tile (1024 bytes each)
BCW = BPR // BC                 # 1024


NKC = 4                         # k-chunks for pipelined dequant
KKC = KT // NKC                 # 8 k-tiles per chunk
KCW = IN_F // NKC               # 1024 k values per chunk
BCC = BPR // NKC                # 512 packed bytes per chunk
NBC = BCC // 32                 # 16 quant blocks per chunk (per row)


def _build(tokens=TOKENS):
    nc = bacc.Bacc("TRN2", target_bir_lowering=False, debug=False,
                   enable_asserts=False)

    x = nc.dram_tensor("x", [tokens, IN_F], F16, kind="ExternalInput").ap()
    qw = nc.dram_tensor("qw", [O_C, BPR], I32, kind="ExternalInput").ap()
    qam = nc.dram_tensor("qam", [O_C, 64], I32, kind="ExternalInput").ap()
    qcode = nc.dram_tensor("qcode", [O_C, 64], F32, kind="ExternalInput").ap()
    qoff = nc.dram_tensor("qoff", [O_C, 64], F32, kind="ExternalInput").ap()
    am2 = nc.dram_tensor("am2", [O_C, 16], F32, kind="ExternalInput").ap()
    c2 = nc.dram_tensor("c2", [O_C, 16], F32, kind="ExternalInput").ap()
    out = nc.dram_tensor("out", [tokens, O_C], F16, kind="ExternalOutput").ap()

    RTB = 256
    NRB = min(4, tokens // RTB)          # ramp blocks
    n_steady = (tokens - NRB * RTB) // TB

    with tile.TileContext(nc) as tc:
        with tc.tile_pool(name="wt_pool", bufs=1) as wt_pool, \
             tc.tile_pool(name="wdram", bufs=1, space="DRAM") as wdram, \
             tc.tile_pool(name="sc_pool", bufs=1) as sc_pool, \
             tc.tile_pool(name="dq", bufs=2) as dq, \
             tc.tile_pool(name="xt_pool", bufs=2) as xt_pool, \
             tc.tile_pool(name="ps_pool", bufs=8, space="PSUM") as ps_pool, \
             tc.tile_pool(name="ob_pool", bufs=2) as ob_pool:
            wts = [wt_pool.tile([P, KKC, O_C], F16, name=f"wt{kc}")
                   for kc in range(NKC)]
            wds = [wdram.tile([O_C, KCW], F16, name=f"wd{kc}")
                   for kc in range(NKC)]

            # ---- scale prep, batched; all small loads via SWDGE (gpsimd)
            # so they are NOT blocked by xbar transposes.
            am3 = sc_pool.tile([P, NB_O, 64], F32, name="am3")
            nc.gpsimd.dma_start(am3, qam.rearrange("(a p) c -> p a c", p=P))
            cd3 = sc_pool.tile([P, NB_O, 64], F32, name="cd3")
            nc.gpsimd.dma_start(cd3, qcode.rearrange("(a p) c -> p a c", p=P))
            of3 = sc_pool.tile([P, NB_O, 64], F32, name="of3")
            nc.gpsimd.dma_start(of3, qoff.rearrange("(a p) c -> p a c", p=P))
            am23 = sc_pool.tile([P, NB_O, 16], F32, name="am23")
            nc.gpsimd.dma_start(am23, am2.rearrange("(a p) c -> p a c", p=P))
            c23 = sc_pool.tile([P, NB_O, 16], F32, name="c23")
            nc.gpsimd.dma_start(c23, c2.rearrange("(a p) c -> p a c", p=P))

            rc = sc_pool.tile([P, NB_O, 64], F32, name="rc")
            nc.vector.reciprocal(rc, cd3)
            s1 = sc_pool.tile([P, NB_O, 64], F32, name="s1")
            nc.vector.tensor_tensor(s1, am3, rc, Alu.mult)
            rc2 = sc_pool.tile([P, NB_O, 16], F32, name="rc2")
            nc.vector.reciprocal(rc2, c23)
            s2 = sc_pool.tile([P, NB_O, 16], F32, name="s2")
            nc.vector.tensor_tensor(s2, am23, rc2, Alu.mult)
            S3 = sc_pool.tile([P, NB_O, 64], F32, name="S3")
            nc.vector.tensor_tensor(
                S3, s1, s2.unsqueeze(3).broadcast_to([P, NB_O, 16, 4]), Alu.mult)
            offS3 = sc_pool.tile([P, NB_O, 64], F32, name="offS3")
            nc.vector.tensor_tensor(offS3, of3, S3, Alu.mult)

            # ---- ramp x transposes (xbar) ----
            xtr, xtr_insts = [], []
            for rb in range(NRB):
                t = xt_pool.tile([P, KT, RTB], F16, name=f"xtr{rb}", bufs=1)
                ti = nc.scalar.dma_start(
                    out=t, in_=x[rb * RTB:(rb + 1) * RTB, :], transpose=True)
                xtr.append(t)
                xtr_insts.append(ti)

            # ---- dequant, k-chunk major; qw loads prefetched via SWDGE
            # with 2-chunk lookahead so stores never head-of-line-block them.
            qts = {}

            def load_chunk(kc):
                bs = slice(kc * BCC, (kc + 1) * BCC)
                for ot in range(NB_O):
                    rs = slice(ot * P, (ot + 1) * P)
                    qt = dq.tile([P, BCC], I32, name="qt", bufs=8)
                    nc.gpsimd.dma_start(qt, qw[rs, bs])
                    qts[(kc, ot)] = qt

            load_chunk(0)
            load_chunk(1)
            wt_insts = []
            for kc in range(NKC):
                if kc + 2 < NKC:
                    load_chunk(kc + 2)
                for ot in range(NB_O):
                    rs = slice(ot * P, (ot + 1) * P)
                    qt = qts.pop((kc, ot))
                    hi = dq.tile([P, BCC], I32, name="hi")
                    nc.vector.tensor_scalar(hi, qt, 4, None,
                                            Alu.logical_shift_right)
                    lo = dq.tile([P, BCC], F32, name="lo")
                    nc.vector.scalar_tensor_tensor(
                        lo, hi, -16.0, qt, Alu.mult, Alu.add)
                    S_b = S3[:, ot, kc * NBC:(kc + 1) * NBC] \
                        .unsqueeze(2).broadcast_to([P, NBC, 32])
                    offS_b = offS3[:, ot, kc * NBC:(kc + 1) * NBC] \
                        .unsqueeze(2).broadcast_to([P, NBC, 32])
                    we = dq.tile([P, BCC], F32, name="we")
                    nc.vector.tensor_tensor(we, lo, S_b, Alu.mult)
                    wo = dq.tile([P, BCC], F32, name="wo")
                    nc.vector.tensor_tensor(wo, hi, S_b, Alu.mult)
                    w_nat = dq.tile([P, KCW], F16, name="w_nat")
                    nc.vector.tensor_tensor(
                        w_nat[:, 0::2], we, offS_b, Alu.subtract)
                    nc.vector.tensor_tensor(
                        w_nat[:, 1::2], wo, offS_b, Alu.subtract)
                    nc.gpsimd.dma_start(wds[kc][rs, :], w_nat)
                wi = nc.scalar.dma_start(out=wts[kc], in_=wds[kc][:, :],
                                         transpose=True)
                wt_insts.append(wi)

            # ---- ramp matmuls: chunk-major across all ramp groups ----
            rps = [[ps_pool.tile([P, O_C], F32, name="ps")
                    for st in range(RTB // P)] for rb in range(NRB)]
            for kc in range(NKC):
                for rb in range(NRB):
                    for st in range(RTB // P):
                        for j in range(KKC):
                            kk = kc * KKC + j
                            nc.tensor.matmul(
                                rps[rb][st],
                                xtr[rb][:, kk, st * P:(st + 1) * P],
                                wts[kc][:, j, :],
                                start=(kk == 0),
                                stop=(kk == KT - 1),
                            )
            for rb in range(NRB):
                for st in range(RTB // P):
                    ob = ob_pool.tile([P, O_C], F16, name="ob")
                    nc.vector.tensor_copy(ob, rps[rb][st])
                    r0 = rb * RTB + st * P
                    nc.gpsimd.dma_start(out[r0:r0 + P, :], ob)

            # ---- steady blocks ----
            base = NRB * RTB
            first_steady_inst = None
            for tb in range(n_steady):
                xt = xt_pool.tile([P, KT, TB], F16, name="xt")
                xi = nc.scalar.dma_start(
                    out=xt, in_=x[base + tb * TB: base + (tb + 1) * TB, :],
                    transpose=True)
                if tb == 0:
                    first_steady_inst = xi
                for st in range(TB // P):
                    ps = ps_pool.tile([P, O_C], F32, name="ps")
                    for kk in range(KT):
                        nc.tensor.matmul(
                            ps,
                            xt[:, kk, st * P:(st + 1) * P],
                            wts[kk // KKC][:, kk % KKC, :],
                            start=(kk == 0),
                            stop=(kk == KT - 1),
                        )
                    ob = ob_pool.tile([P, O_C], F16, name="ob")
                    nc.vector.tensor_copy(ob, ps)
                    r0 = base + tb * TB + st * P
                    nc.gpsimd.dma_start(out[r0:r0 + P, :], ob)

            # ---- pin xbar order: xtr0, wt0, xtr1, wt1, ..., first steady xt
            if len(xtr_insts) == 4:
                chain = [xtr_insts[0], xtr_insts[1], wt_insts[0],
                         xtr_insts[2], xtr_insts[3],
                         wt_insts[1], wt_insts[2], wt_insts[3]]
            else:
                chain = []
                for i in range(max(len(xtr_insts), len(wt_insts))):
                    if i < len(xtr_insts):
                        chain.append(xtr_insts[i])
                    if i < len(wt_insts):
                        chain.append(wt_insts[i])
            if first_steady_inst is not None:
                chain.append(first_steady_inst)
            for a, b in zip(chain[1:], chain):
                tile_rust_add_dep(a.ins, b.ins, True, "xbar order")

    nc.compile()
    return nc


_NC_CACHE = {}


def _get_nc(tokens=TOKENS):
    if tokens not in _NC_CACHE:
        _NC_CACHE[tokens] = _build(tokens)
    return _NC_CACHE[tokens]


def _shard(inputs):
    x = np.ascontiguousarray(np.asarray(inputs["x"], dtype=np.float16))
    qw = np.asarray(inputs["quantized_weight"], dtype=np.int32)
    qam = np.asarray(inputs["quant_absmax"], dtype=np.int32)
    qcode = np.asarray(inputs["quant_code"], dtype=np.float32)
    qoff = np.asarray(inputs["quant_offset"], dtype=np.float32)
    am2 = np.asarray(inputs["state2_absmax"], dtype=np.float32)
    c2 = np.asarray(inputs["state2_code"], dtype=np.float32)

    pb = O_C * BPR        # packed bytes per core
    nb1 = O_C * 64        # primary blocks per core
    nb2 = O_C * 16        # secondary blocks per core
    in_maps = []
    for c in range(N_CORES):
        in_maps.append({
            "x": x,
            "qw": np.ascontiguousarray(
                qw[c * pb:(c + 1) * pb].reshape(O_C, BPR)),
            "qam": np.ascontiguousarray(
                qam[c * nb1:(c + 1) * nb1].reshape(O_C, 64)),
            "qcode": np.ascontiguousarray(
                qcode[c * nb1:(c + 1) * nb1].reshape(O_C, 64)),
            "qoff": np.ascontiguousarray(
                qoff[c * nb1:(c + 1) * nb1].reshape(O_C, 64)),
            "am2": np.ascontiguousarray(
                am2[c * nb2:(c + 1) * nb2].reshape(O_C, 16)),
            "c2": np.ascontiguousarray(
                c2[c * nb2:(c + 1) * nb2].reshape(O_C, 16)),
        })
    return in_maps


def _run(inputs, trace=False, trace_cores=None):
    nc = _get_nc()
    in_maps = _shard(inputs)
    res = run_bass_kernel_spmd(
        nc, in_maps, list(range(N_CORES)), trace=trace,
        trace_cores=trace_cores)
    out = np.concatenate([r["out"] for r in res.results], axis=1)
    return out, res


def kernel(**inputs) -> np.ndarray:
    out, _ = _run(inputs, trace=False)
    return out



# revision 4
# speedup vs baseline: 1.0234x; 1.0234x over previous
"""NF4-style 4-bit quantized linear: out = x @ dequant(w).T on 8 TRN2 NeuronCores.

Column-parallel sharding: core c owns output features [c*512, (c+1)*512) and the
matching slices of the packed weight + quant state arrays; x is replicated.

Per core:
  1. dequantize the 512x4096 weight slice on-chip (DVE, u8/f16 ops) in
     progressive k-chunks, round-tripping each chunk through DRAM with an xbar
     transpose to [k-partition, outf] layout,
  2. stream x through xbar transpose DMAs ([token, k] -> [k, token]) with two
     small 128-token lead blocks, and run the fp16 matmul on the PE array,
     accumulating in PSUM over 32 k-tiles.

Queue discipline (critical for the startup ramp):
  - ALL xbar transposes ride the ACT (scalar) HWDGE ring: concurrent
    transposes on the two HWDGE rings corrupt data (shared xbar), so they
    must be on one ring. Emission order: xtr0, xtr1, all W chunks, rest.
  - ALL plain DMAs (packed-scale load, packed-weight load, w-chunk stores,
    output writes) ride the SP (sync) HWDGE ring; plain HWDGE DMAs overlap
    in-flight transposes, unlike SWDGE (gpsimd) DMAs which Tile serializes
    against them. No gpsimd DMAs anywhere.
Host packs the five quant-state arrays into one f32 tensor (one DMA) and
provides qw as uint8; host gathers per-core outputs by concat along axis 1.
"""
import numpy as np

import concourse.bass as bass
import concourse.mybir as mybir
import concourse.tile as tile
from concourse import bacc
from concourse.bass_utils import run_bass_kernel_spmd

F16 = mybir.dt.float16
F32 = mybir.dt.float32
U8 = mybir.dt.uint8
Alu = mybir.AluOpType

P = 128
TOKENS = 8192
IN_F = 4096
OUT_F = 4096
N_CORES = 8
O_C = OUT_F // N_CORES          # 512 out features per core
KT = IN_F // P                  # 32 k-tiles
BPR = IN_F // 2                 # 2048 packed bytes per weight row
NB_O = O_C // P                 # 4 o-tiles of 128 rows

CHUNKS = [1, 1, 2, 4, 8, 8, 8]  # k-tiles per dequant chunk (progressive)
KOFF = [0, 1, 2, 4, 8, 16, 24]  # k-tile offset of each chunk
X_BLOCKS = [128, 128, 256] + [512] * 15   # token blocks
N_RAMP = 3


def _build(tokens=TOKENS):
    nc = bacc.Bacc("TRN2", target_bir_lowering=False, debug=False,
                   enable_asserts=False)

    x = nc.dram_tensor("x", [tokens, IN_F], F16, kind="ExternalInput").ap()
    qw = nc.dram_tensor("qw", [O_C, BPR], U8, kind="ExternalInput").ap()
    # packed quant state: [qam | qcode | qoff | am2 | c2] along columns
    scp = nc.dram_tensor("scp", [O_C, 224], F32, kind="ExternalInput").ap()
    out = nc.dram_tensor("out", [tokens, O_C], F16, kind="ExternalOutput").ap()

    kk2chunk = []
    for ci, c in enumerate(CHUNKS):
        for j in range(c):
            kk2chunk.append((ci, j))

    with tile.TileContext(nc) as tc:
        with tc.tile_pool(name="wt_pool", bufs=1) as wt_pool, \
             tc.tile_pool(name="wdram", bufs=1, space="DRAM") as wdram, \
             tc.tile_pool(name="sc_pool", bufs=1) as sc_pool, \
             tc.tile_pool(name="dq", bufs=1) as dq, \
             tc.tile_pool(name="xt_pool", bufs=2) as xt_pool, \
             tc.tile_pool(name="ps_pool", bufs=8, space="PSUM") as ps_pool, \
             tc.tile_pool(name="ob_pool", bufs=2) as ob_pool:
            wts = [wt_pool.tile([P, c, O_C], F16, name=f"wt{ci}")
                   for ci, c in enumerate(CHUNKS)]
            wds = [wdram.tile([O_C, c * P], F16, name=f"wd{ci}")
                   for ci, c in enumerate(CHUNKS)]

            # ---- plain loads (SP ring): packed scales + full packed weights
            sc3 = sc_pool.tile([P, NB_O, 224], F32, name="sc3")
            nc.sync.dma_start(sc3, scp.rearrange("(a p) c -> p a c", p=P))
            qt_all = dq.tile([P, NB_O, BPR], U8, name="qt_all")
            nc.sync.dma_start(qt_all, qw.rearrange("(a p) c -> p a c", p=P))

            # ---- first two x transposes (ACT ring) before anything else
            xtiles, xinsts = [], []
            r0 = 0
            for bi, tb in enumerate(X_BLOCKS[:2]):
                t = xt_pool.tile([P, KT, tb], F16, name=f"xtr{bi}", bufs=1)
                xinsts.append(nc.scalar.dma_start(out=t, in_=x[r0:r0 + tb, :],
                                                  transpose=True))
                xtiles.append(t)
                r0 += tb

            # ---- scale math on DVE -> f16 scale/offset tiles
            am3 = sc3[:, :, 0:64]
            cd3 = sc3[:, :, 64:128]
            of3 = sc3[:, :, 128:192]
            am23 = sc3[:, :, 192:208]
            c23 = sc3[:, :, 208:224]
            rc = sc_pool.tile([P, NB_O, 64], F32, name="rc")
            nc.vector.reciprocal(rc, cd3)
            s1 = sc_pool.tile([P, NB_O, 64], F32, name="s1")
            nc.vector.tensor_tensor(s1, am3, rc, Alu.mult)
            rc2 = sc_pool.tile([P, NB_O, 16], F32, name="rc2")
            nc.vector.reciprocal(rc2, c23)
            s2 = sc_pool.tile([P, NB_O, 16], F32, name="s2")
            nc.vector.tensor_tensor(s2, am23, rc2, Alu.mult)
            S3 = sc_pool.tile([P, NB_O, 64], F32, name="S3")
            nc.vector.tensor_tensor(
                S3, s1, s2.unsqueeze(3).broadcast_to([P, NB_O, 16, 4]), Alu.mult)
            offS3 = sc_pool.tile([P, NB_O, 64], F32, name="offS3")
            nc.vector.tensor_tensor(offS3, of3, S3, Alu.mult)
            S3h = sc_pool.tile([P, NB_O, 64], F16, name="S3h")
            nc.vector.tensor_copy(S3h, S3)
            offS3h = sc_pool.tile([P, NB_O, 64], F16, name="offS3h")
            nc.vector.tensor_copy(offS3h, offS3)

            # ---- dequant chunks: DVE -> store (SP) -> transpose (ACT)
            for ci, c in enumerate(CHUNKS):
                bcc = 64 * c
                nbc = 2 * c
                b0 = KOFF[ci] * 64
                qt = qt_all[:, :, b0:b0 + bcc]
                hi = dq.tile([P, NB_O, bcc], U8, name="hi")
                nc.vector.tensor_scalar(hi, qt, 4, None,
                                        Alu.logical_shift_right)
                lo = dq.tile([P, NB_O, bcc], U8, name="lo")
                nc.vector.tensor_scalar(lo, qt, 15, None, Alu.bitwise_and)
                bsl = slice(KOFF[ci] * 2, KOFF[ci] * 2 + nbc)
                S_b = S3h[:, :, bsl].unsqueeze(3) \
                    .broadcast_to([P, NB_O, nbc, 32])
                offS_b = offS3h[:, :, bsl].unsqueeze(3) \
                    .broadcast_to([P, NB_O, nbc, 32])
                we = dq.tile([P, NB_O, bcc], F16, name="we")
                nc.vector.tensor_tensor(we, lo, S_b, Alu.mult)
                wo = dq.tile([P, NB_O, bcc], F16, name="wo")
                nc.vector.tensor_tensor(wo, hi, S_b, Alu.mult)
                w_nat = dq.tile([P, NB_O, 128 * c], F16, name="w_nat", bufs=2)
                nc.vector.tensor_tensor(
                    w_nat[:, :, 0::2], we, offS_b, Alu.subtract)
                nc.vector.tensor_tensor(
                    w_nat[:, :, 1::2], wo, offS_b, Alu.subtract)
                nc.sync.dma_start(
                    wds[ci].rearrange("(a p) c -> p a c", p=P), w_nat)
                nc.scalar.dma_start(out=wts[ci], in_=wds[ci][:, :],
                                    transpose=True)

            # ---- third x block (256 tokens)
            t = xt_pool.tile([P, KT, X_BLOCKS[2]], F16, name="xtr2", bufs=1)
            xinsts.append(nc.scalar.dma_start(
                out=t, in_=x[r0:r0 + X_BLOCKS[2], :], transpose=True))
            xtiles.append(t)
            r0 += X_BLOCKS[2]

            def evac(ps, row0):
                ob = ob_pool.tile([P, O_C], F16, name="ob")
                nc.vector.tensor_copy(ob, ps)
                nc.sync.dma_start(out[row0:row0 + P, :], ob)

            # wave A: blocks 0,1 interleaved, kk-outer (2 MMs per arriving
            # k-tile keeps the PE fed while W chunks trickle in).
            psA = [ps_pool.tile([P, O_C], F32, name="ps") for _ in range(2)]
            for kk in range(KT):
                ci, j = kk2chunk[kk]
                for b in (0, 1):
                    nc.tensor.matmul(
                        psA[b], xtiles[b][:, kk, 0:P], wts[ci][:, j, :],
                        start=(kk == 0), stop=(kk == KT - 1))
            # wave B: block 2, kk-outer
            psB = [ps_pool.tile([P, O_C], F32, name="ps") for _ in range(2)]
            for kk in range(KT):
                ci, j = kk2chunk[kk]
                for st in range(2):
                    nc.tensor.matmul(
                        psB[st], xtiles[2][:, kk, st * P:(st + 1) * P],
                        wts[ci][:, j, :],
                        start=(kk == 0), stop=(kk == KT - 1))
            for b in (0, 1):
                evac(psA[b], b * P)
            for st in range(2):
                evac(psB[st], 256 + st * P)

            # steady blocks: st-outer, kk-inner (W fully resident)
            row0 = 512
            for bi in range(N_RAMP, len(X_BLOCKS)):
                tb = X_BLOCKS[bi]
                xt = xt_pool.tile([P, KT, tb], F16, name="xt")
                xinsts.append(nc.scalar.dma_start(
                    out=xt, in_=x[r0:r0 + tb, :], transpose=True))
                r0 += tb
                for st in range(tb // P):
                    ps = ps_pool.tile([P, O_C], F32, name="ps")
                    for kk in range(KT):
                        ci, j = kk2chunk[kk]
                        nc.tensor.matmul(
                            ps, xt[:, kk, st * P:(st + 1) * P],
                            wts[ci][:, j, :],
                            start=(kk == 0), stop=(kk == KT - 1))
                    evac(ps, row0)
                    row0 += P

    nc.compile()
    return nc


_NC_CACHE = {}


def _get_nc(tokens=TOKENS):
    if tokens not in _NC_CACHE:
        _NC_CACHE[tokens] = _build(tokens)
    return _NC_CACHE[tokens]


def _shard(inputs):
    x = np.ascontiguousarray(np.asarray(inputs["x"], dtype=np.float16))
    qw = np.asarray(inputs["quantized_weight"], dtype=np.int32).astype(np.uint8)
    qam = np.asarray(inputs["quant_absmax"], dtype=np.int32).astype(np.float32)
    qcode = np.asarray(inputs["quant_code"], dtype=np.float32)
    qoff = np.asarray(inputs["quant_offset"], dtype=np.float32)
    am2 = np.asarray(inputs["state2_absmax"], dtype=np.float32)
    c2 = np.asarray(inputs["state2_code"], dtype=np.float32)

    pb = O_C * BPR        # packed bytes per core
    nb1 = O_C * 64        # primary blocks per core
    nb2 = O_C * 16        # secondary blocks per core
    in_maps = []
    for c in range(N_CORES):
        scp = np.concatenate([
            qam[c * nb1:(c + 1) * nb1].reshape(O_C, 64),
            qcode[c * nb1:(c + 1) * nb1].reshape(O_C, 64),
            qoff[c * nb1:(c + 1) * nb1].reshape(O_C, 64),
            am2[c * nb2:(c + 1) * nb2].reshape(O_C, 16),
            c2[c * nb2:(c + 1) * nb2].reshape(O_C, 16),
        ], axis=1)
        in_maps.append({
            "x": x,
            "qw": np.ascontiguousarray(
                qw[c * pb:(c + 1) * pb].reshape(O_C, BPR)),
            "scp": np.ascontiguousarray(scp),
        })
    return in_maps


def _run(inputs, trace=False, trace_cores=None):
    nc = _get_nc()
    in_maps = _shard(inputs)
    res = run_bass_kernel_spmd(
        nc, in_maps, list(range(N_CORES)), trace=trace,
        trace_cores=trace_cores)
    out = np.concatenate([r["out"] for r in res.results], axis=1)
    return out, res


def kernel(**inputs) -> np.ndarray:
    out, _ = _run(inputs, trace=False)
    return out


# revision 9
# speedup vs baseline: 1.0308x; 1.0072x over previous
"""NF4-style 4-bit quantized linear: out = x @ dequant(w).T on 8 TRN2 NeuronCores.

Column-parallel sharding: core c owns output features [c*512, (c+1)*512) and the
matching slices of the packed weight + quant state arrays; x is replicated.

Per core:
  1. dequantize the 512x4096 weight slice on-chip (DVE, u8/f16 ops) in
     progressive k-chunks, round-tripping each chunk through DRAM with an xbar
     transpose to [k-partition, outf] layout,
  2. stream x through xbar transpose DMAs ([token, k] -> [k, token]) with two
     small 128-token lead blocks, and run the fp16 matmul on the PE array,
     accumulating in PSUM over 32 k-tiles.

Queue discipline (critical for the startup ramp):
  - ALL xbar transposes ride the ACT (scalar) HWDGE ring: concurrent
    transposes on the two HWDGE rings corrupt data (shared xbar), so they
    must be on one ring. Emission order: xtr0, xtr1, all W chunks, rest.
  - ALL plain DMAs (packed-scale load, packed-weight load, w-chunk stores,
    output writes) ride the SP (sync) HWDGE ring; plain HWDGE DMAs overlap
    in-flight transposes, unlike SWDGE (gpsimd) DMAs which Tile serializes
    against them. No gpsimd DMAs anywhere.
Host packs the five quant-state arrays into one f32 tensor (one DMA) and
provides qw as uint8; host gathers per-core outputs by concat along axis 1.
"""
import numpy as np

import concourse.bass as bass
import concourse.mybir as mybir
import concourse.tile as tile
from concourse import bacc
from concourse.tile_rust import add_dep_helper as tile_rust_add_dep
from concourse.bass_utils import run_bass_kernel_spmd

F16 = mybir.dt.float16
F32 = mybir.dt.float32
U8 = mybir.dt.uint8
Alu = mybir.AluOpType

P = 128
TOKENS = 8192
IN_F = 4096
OUT_F = 4096
N_CORES = 8
O_C = OUT_F // N_CORES          # 512 out features per core
KT = IN_F // P                  # 32 k-tiles
BPR = IN_F // 2                 # 2048 packed bytes per weight row
NB_O = O_C // P                 # 4 o-tiles of 128 rows

CHUNKS = [1, 1, 2, 4, 8, 8, 8]  # k-tiles per dequant chunk (progressive)
KOFF = [0, 1, 2, 4, 8, 16, 24]  # k-tile offset of each chunk
X_BLOCKS = [128, 128, 256] + [512] * 15   # token blocks
N_RAMP = 3


def _build(tokens=TOKENS):
    nc = bacc.Bacc("TRN2", target_bir_lowering=False, debug=False,
                   enable_asserts=False)

    x = nc.dram_tensor("x", [tokens, IN_F], F16, kind="ExternalInput").ap()
    qw = nc.dram_tensor("qw", [O_C, BPR], U8, kind="ExternalInput").ap()
    # packed quant state: [qam | qcode | qoff | am2 | c2] along columns
    scp = nc.dram_tensor("scp", [O_C, 224], F32, kind="ExternalInput").ap()
    out = nc.dram_tensor("out", [tokens, O_C], F16, kind="ExternalOutput").ap()

    kk2chunk = []
    for ci, c in enumerate(CHUNKS):
        for j in range(c):
            kk2chunk.append((ci, j))

    with tile.TileContext(nc) as tc:
        with tc.tile_pool(name="wt_pool", bufs=1) as wt_pool, \
             tc.tile_pool(name="wdram", bufs=1, space="DRAM") as wdram, \
             tc.tile_pool(name="sc_pool", bufs=1) as sc_pool, \
             tc.tile_pool(name="dq", bufs=1) as dq, \
             tc.tile_pool(name="xt_pool", bufs=2) as xt_pool, \
             tc.tile_pool(name="ps_pool", bufs=8, space="PSUM") as ps_pool, \
             tc.tile_pool(name="ob_pool", bufs=2) as ob_pool:
            wts = [wt_pool.tile([P, c, O_C], F16, name=f"wt{ci}")
                   for ci, c in enumerate(CHUNKS)]
            wds = [wdram.tile([O_C, c * P], F16, name=f"wd{ci}")
                   for ci, c in enumerate(CHUNKS)]

            # Effectively all DMA traffic serializes against in-flight xbar
            # transposes, so pin ONE explicit global order tuned for the ramp:
            # loads, xtr0, xtr1, (store_i, wt_i)*, xtr2, steady xts.
            chain = []

            # ---- plain loads (SP ring): packed scales + full packed weights
            sc3 = sc_pool.tile([P, NB_O, 224], F32, name="sc3")
            chain.append(nc.sync.dma_start(
                sc3, scp.rearrange("(a p) c -> p a c", p=P)))
            qt_all = dq.tile([P, NB_O, BPR], U8, name="qt_all")
            chain.append(nc.sync.dma_start(
                qt_all, qw.rearrange("(a p) c -> p a c", p=P)))

            # ---- first two x transposes (ACT ring) before anything else
            xtiles, xinsts = [], []
            r0 = 0
            for bi, tb in enumerate(X_BLOCKS[:2]):
                t = xt_pool.tile([P, KT, tb], F16, name=f"xtr{bi}", bufs=1)
                xi = nc.scalar.dma_start(out=t, in_=x[r0:r0 + tb, :],
                                         transpose=True)
                xinsts.append(xi)
                chain.append(xi)
                xtiles.append(t)
                r0 += tb

            # ---- scale math on DVE -> f16 scale/offset tiles
            am3 = sc3[:, :, 0:64]
            cd3 = sc3[:, :, 64:128]
            of3 = sc3[:, :, 128:192]
            am23 = sc3[:, :, 192:208]
            c23 = sc3[:, :, 208:224]
            rc = sc_pool.tile([P, NB_O, 64], F32, name="rc")
            nc.vector.reciprocal(rc, cd3)
            s1 = sc_pool.tile([P, NB_O, 64], F32, name="s1")
            nc.vector.tensor_tensor(s1, am3, rc, Alu.mult)
            rc2 = sc_pool.tile([P, NB_O, 16], F32, name="rc2")
            nc.vector.reciprocal(rc2, c23)
            s2 = sc_pool.tile([P, NB_O, 16], F32, name="s2")
            nc.vector.tensor_tensor(s2, am23, rc2, Alu.mult)
            S3 = sc_pool.tile([P, NB_O, 64], F32, name="S3")
            nc.vector.tensor_tensor(
                S3, s1, s2.unsqueeze(3).broadcast_to([P, NB_O, 16, 4]), Alu.mult)
            offS3 = sc_pool.tile([P, NB_O, 64], F32, name="offS3")
            nc.vector.tensor_tensor(offS3, of3, S3, Alu.mult)
            S3h = sc_pool.tile([P, NB_O, 64], F16, name="S3h")
            nc.vector.tensor_copy(S3h, S3)
            offS3h = sc_pool.tile([P, NB_O, 64], F16, name="offS3h")
            nc.vector.tensor_copy(offS3h, offS3)

            # ---- dequant chunks: DVE -> store (SP) -> transpose (ACT)
            for ci, c in enumerate(CHUNKS):
                bcc = 64 * c
                nbc = 2 * c
                b0 = KOFF[ci] * 64
                qt = qt_all[:, :, b0:b0 + bcc]
                hi = dq.tile([P, NB_O, bcc], U8, name="hi")
                nc.vector.tensor_scalar(hi, qt, 4, None,
                                        Alu.logical_shift_right)
                lo = dq.tile([P, NB_O, bcc], U8, name="lo")
                nc.vector.tensor_scalar(lo, qt, 15, None, Alu.bitwise_and)
                bsl = slice(KOFF[ci] * 2, KOFF[ci] * 2 + nbc)
                S_b = S3h[:, :, bsl].unsqueeze(3) \
                    .broadcast_to([P, NB_O, nbc, 32])
                offS_b = offS3h[:, :, bsl].unsqueeze(3) \
                    .broadcast_to([P, NB_O, nbc, 32])
                we = dq.tile([P, NB_O, bcc], F16, name="we")
                nc.vector.tensor_tensor(we, lo, S_b, Alu.mult)
                wo = dq.tile([P, NB_O, bcc], F16, name="wo")
                nc.vector.tensor_tensor(wo, hi, S_b, Alu.mult)
                w_nat = dq.tile([P, NB_O, 128 * c], F16, name="w_nat", bufs=4)
                nc.vector.tensor_tensor(
                    w_nat[:, :, 0::2], we, offS_b, Alu.subtract)
                nc.vector.tensor_tensor(
                    w_nat[:, :, 1::2], wo, offS_b, Alu.subtract)
                chain.append(nc.sync.dma_start(
                    wds[ci].rearrange("(a p) c -> p a c", p=P), w_nat))
                chain.append(nc.scalar.dma_start(out=wts[ci],
                                                 in_=wds[ci][:, :],
                                                 transpose=True))

            # ---- third x block (256 tokens)
            t = xt_pool.tile([P, KT, X_BLOCKS[2]], F16, name="xtr2", bufs=1)
            xi = nc.scalar.dma_start(
                out=t, in_=x[r0:r0 + X_BLOCKS[2], :], transpose=True)
            xinsts.append(xi)
            chain.append(xi)
            xtiles.append(t)
            r0 += X_BLOCKS[2]

            def evac(ps, row0):
                ob = ob_pool.tile([P, O_C], F16, name="ob")
                nc.vector.tensor_copy(ob, ps)
                nc.sync.dma_start(out[row0:row0 + P, :], ob)

            # wave A: blocks 0,1 interleaved, kk-outer (2 MMs per arriving
            # k-tile keeps the PE fed while W chunks trickle in).
            psA = [ps_pool.tile([P, O_C], F32, name="ps") for _ in range(2)]
            for kk in range(KT):
                ci, j = kk2chunk[kk]
                for b in (0, 1):
                    nc.tensor.matmul(
                        psA[b], xtiles[b][:, kk, 0:P], wts[ci][:, j, :],
                        start=(kk == 0), stop=(kk == KT - 1))
            # wave B: block 2, kk-outer
            psB = [ps_pool.tile([P, O_C], F32, name="ps") for _ in range(2)]
            for kk in range(KT):
                ci, j = kk2chunk[kk]
                for st in range(2):
                    nc.tensor.matmul(
                        psB[st], xtiles[2][:, kk, st * P:(st + 1) * P],
                        wts[ci][:, j, :],
                        start=(kk == 0), stop=(kk == KT - 1))
            for b in (0, 1):
                evac(psA[b], b * P)
            for st in range(2):
                evac(psB[st], 256 + st * P)

            # steady blocks: st-outer, kk-inner (W fully resident)
            row0 = 512
            for bi in range(N_RAMP, len(X_BLOCKS)):
                tb = X_BLOCKS[bi]
                xt = xt_pool.tile([P, KT, tb], F16, name="xt")
                xi = nc.scalar.dma_start(
                    out=xt, in_=x[r0:r0 + tb, :], transpose=True)
                xinsts.append(xi)
                chain.append(xi)
                r0 += tb
                for st in range(tb // P):
                    ps = ps_pool.tile([P, O_C], F32, name="ps")
                    for kk in range(KT):
                        ci, j = kk2chunk[kk]
                        nc.tensor.matmul(
                            ps, xt[:, kk, st * P:(st + 1) * P],
                            wts[ci][:, j, :],
                            start=(kk == 0), stop=(kk == KT - 1))
                    evac(ps, row0)
                    row0 += P

            for a, b in zip(chain[1:], chain):
                tile_rust_add_dep(a.ins, b.ins, True, "global dma order")

    nc.compile()
    return nc


_NC_CACHE = {}


def _get_nc(tokens=TOKENS):
    if tokens not in _NC_CACHE:
        _NC_CACHE[tokens] = _build(tokens)
    return _NC_CACHE[tokens]


def _shard(inputs):
    x = np.ascontiguousarray(np.asarray(inputs["x"], dtype=np.float16))
    qw = np.asarray(inputs["quantized_weight"], dtype=np.int32).astype(np.uint8)
    qam = np.asarray(inputs["quant_absmax"], dtype=np.int32).astype(np.float32)
    qcode = np.asarray(inputs["quant_code"], dtype=np.float32)
    qoff = np.asarray(inputs["quant_offset"], dtype=np.float32)
    am2 = np.asarray(inputs["state2_absmax"], dtype=np.float32)
    c2 = np.asarray(inputs["state2_code"], dtype=np.float32)

    pb = O_C * BPR        # packed bytes per core
    nb1 = O_C * 64        # primary blocks per core
    nb2 = O_C * 16        # secondary blocks per core
    in_maps = []
    for c in range(N_CORES):
        scp = np.concatenate([
            qam[c * nb1:(c + 1) * nb1].reshape(O_C, 64),
            qcode[c * nb1:(c + 1) * nb1].reshape(O_C, 64),
            qoff[c * nb1:(c + 1) * nb1].reshape(O_C, 64),
            am2[c * nb2:(c + 1) * nb2].reshape(O_C, 16),
            c2[c * nb2:(c + 1) * nb2].reshape(O_C, 16),
        ], axis=1)
        in_maps.append({
            "x": x,
            "qw": np.ascontiguousarray(
                qw[c * pb:(c + 1) * pb].reshape(O_C, BPR)),
            "scp": np.ascontiguousarray(scp),
        })
    return in_maps


def _run(inputs, trace=False, trace_cores=None):
    nc = _get_nc()
    in_maps = _shard(inputs)
    res = run_bass_kernel_spmd(
        nc, in_maps, list(range(N_CORES)), trace=trace,
        trace_cores=trace_cores)
    out = np.concatenate([r["out"] for r in res.results], axis=1)
    return out, res


def kernel(**inputs) -> np.ndarray:
    out, _ = _run(inputs, trace=False)
    return out


# revision 10
# speedup vs baseline: 1.0522x; 1.0208x over previous
"""NF4-style 4-bit quantized linear: out = x @ dequant(w).T on 8 TRN2 NeuronCores.

Column-parallel sharding: core c owns output features [c*512, (c+1)*512) and the
matching slices of the packed weight + quant state arrays; x is replicated.

Per core:
  1. dequantize the 512x4096 weight slice on-chip (DVE, u8/f16 ops) in
     progressive k-chunks, round-tripping each chunk through DRAM with an xbar
     transpose to [k-partition, outf] layout,
  2. stream x through xbar transpose DMAs ([token, k] -> [k, token]) with two
     small 128-token lead blocks, and run the fp16 matmul on the PE array,
     accumulating in PSUM over 32 k-tiles.

Queue discipline (critical for the startup ramp):
  - ALL xbar transposes ride the ACT (scalar) HWDGE ring: concurrent
    transposes on the two HWDGE rings corrupt data (shared xbar), so they
    must be on one ring. Emission order: xtr0, xtr1, all W chunks, rest.
  - ALL plain DMAs (packed-scale load, packed-weight load, w-chunk stores,
    output writes) ride the SP (sync) HWDGE ring; plain HWDGE DMAs overlap
    in-flight transposes, unlike SWDGE (gpsimd) DMAs which Tile serializes
    against them. No gpsimd DMAs anywhere.
Host packs the five quant-state arrays into one f32 tensor (one DMA) and
provides qw as uint8; host gathers per-core outputs by concat along axis 1.
"""
import numpy as np

import concourse.bass as bass
import concourse.mybir as mybir
import concourse.tile as tile
from concourse import bacc
from concourse.tile_rust import add_dep_helper as tile_rust_add_dep
from concourse.bass_utils import run_bass_kernel_spmd

F16 = mybir.dt.float16
F32 = mybir.dt.float32
U8 = mybir.dt.uint8
Alu = mybir.AluOpType

P = 128
TOKENS = 8192
IN_F = 4096
OUT_F = 4096
N_CORES = 8
O_C = OUT_F // N_CORES          # 512 out features per core
KT = IN_F // P                  # 32 k-tiles
BPR = IN_F // 2                 # 2048 packed bytes per weight row
NB_O = O_C // P                 # 4 o-tiles of 128 rows

CHUNKS = [1, 1, 2, 4, 8, 8, 8]  # k-tiles per dequant chunk (progressive)
KOFF = [0, 1, 2, 4, 8, 16, 24]  # k-tile offset of each chunk
X_BLOCKS = [128, 128, 256] + [512] * 15   # token blocks
N_RAMP = 3


def _build(tokens=TOKENS):
    nc = bacc.Bacc("TRN2", target_bir_lowering=False, debug=False,
                   enable_asserts=False)

    x = nc.dram_tensor("x", [tokens, IN_F], F16, kind="ExternalInput").ap()
    qw = nc.dram_tensor("qw", [O_C, BPR], U8, kind="ExternalInput").ap()
    # packed quant state: [qam | qcode | qoff | am2 | c2] along columns
    scp = nc.dram_tensor("scp", [O_C, 224], F32, kind="ExternalInput").ap()
    out = nc.dram_tensor("out", [tokens, O_C], F16, kind="ExternalOutput").ap()

    kk2chunk = []
    for ci, c in enumerate(CHUNKS):
        for j in range(c):
            kk2chunk.append((ci, j))

    with tile.TileContext(nc) as tc:
        with tc.tile_pool(name="wt_pool", bufs=1) as wt_pool, \
             tc.tile_pool(name="wdram", bufs=1, space="DRAM") as wdram, \
             tc.tile_pool(name="sc_pool", bufs=1) as sc_pool, \
             tc.tile_pool(name="dq", bufs=1) as dq, \
             tc.tile_pool(name="xt_pool", bufs=2) as xt_pool, \
             tc.tile_pool(name="ps_pool", bufs=8, space="PSUM") as ps_pool, \
             tc.tile_pool(name="ob_pool", bufs=2) as ob_pool:
            wts = [wt_pool.tile([P, c, O_C], F16, name=f"wt{ci}")
                   for ci, c in enumerate(CHUNKS)]
            wds = [wdram.tile([O_C, c * P], F16, name=f"wd{ci}")
                   for ci, c in enumerate(CHUNKS)]

            # Effectively all DMA traffic serializes against in-flight xbar
            # transposes, so pin ONE explicit global order tuned for the ramp:
            # loads, xtr0, xtr1, (store_i, wt_i)*, xtr2, steady xts.
            chain = []

            # ---- first x transpose (ACT ring) leads the chain
            xtiles, xinsts = [], []
            r0 = 0
            t = xt_pool.tile([P, KT, X_BLOCKS[0]], F16, name="xtr0", bufs=1)
            xi = nc.scalar.dma_start(out=t, in_=x[r0:r0 + X_BLOCKS[0], :],
                                     transpose=True)
            xinsts.append(xi)
            chain.append(xi)
            xtiles.append(t)
            r0 += X_BLOCKS[0]

            # ---- plain loads (SP ring): packed scales + full packed weights
            sc3 = sc_pool.tile([P, NB_O, 224], F32, name="sc3")
            chain.append(nc.sync.dma_start(
                sc3, scp.rearrange("(a p) c -> p a c", p=P)))
            qt_all = dq.tile([P, NB_O, BPR], U8, name="qt_all")
            chain.append(nc.sync.dma_start(
                qt_all, qw.rearrange("(a p) c -> p a c", p=P)))

            # ---- second x transpose
            t = xt_pool.tile([P, KT, X_BLOCKS[1]], F16, name="xtr1", bufs=1)
            xi = nc.scalar.dma_start(out=t, in_=x[r0:r0 + X_BLOCKS[1], :],
                                     transpose=True)
            xinsts.append(xi)
            chain.append(xi)
            xtiles.append(t)
            r0 += X_BLOCKS[1]

            # ---- scale math on DVE -> f16 scale/offset tiles
            am3 = sc3[:, :, 0:64]
            cd3 = sc3[:, :, 64:128]
            of3 = sc3[:, :, 128:192]
            am23 = sc3[:, :, 192:208]
            c23 = sc3[:, :, 208:224]
            rc = sc_pool.tile([P, NB_O, 64], F32, name="rc")
            nc.vector.reciprocal(rc, cd3)
            s1 = sc_pool.tile([P, NB_O, 64], F32, name="s1")
            nc.vector.tensor_tensor(s1, am3, rc, Alu.mult)
            rc2 = sc_pool.tile([P, NB_O, 16], F32, name="rc2")
            nc.vector.reciprocal(rc2, c23)
            s2 = sc_pool.tile([P, NB_O, 16], F32, name="s2")
            nc.vector.tensor_tensor(s2, am23, rc2, Alu.mult)
            S3 = sc_pool.tile([P, NB_O, 64], F32, name="S3")
            nc.vector.tensor_tensor(
                S3, s1, s2.unsqueeze(3).broadcast_to([P, NB_O, 16, 4]), Alu.mult)
            offS3 = sc_pool.tile([P, NB_O, 64], F32, name="offS3")
            nc.vector.tensor_tensor(offS3, of3, S3, Alu.mult)
            S3h = sc_pool.tile([P, NB_O, 64], F16, name="S3h")
            nc.vector.tensor_copy(S3h, S3)
            offS3h = sc_pool.tile([P, NB_O, 64], F16, name="offS3h")
            nc.vector.tensor_copy(offS3h, offS3)

            # ---- dequant chunks: DVE -> store (SP) -> transpose (ACT)
            for ci, c in enumerate(CHUNKS):
                bcc = 64 * c
                nbc = 2 * c
                b0 = KOFF[ci] * 64
                qt = qt_all[:, :, b0:b0 + bcc]
                hi = dq.tile([P, NB_O, bcc], U8, name="hi")
                nc.vector.tensor_scalar(hi, qt, 4, None,
                                        Alu.logical_shift_right)
                lo = dq.tile([P, NB_O, bcc], U8, name="lo")
                nc.vector.tensor_scalar(lo, qt, 15, None, Alu.bitwise_and)
                bsl = slice(KOFF[ci] * 2, KOFF[ci] * 2 + nbc)
                S_b = S3h[:, :, bsl].unsqueeze(3) \
                    .broadcast_to([P, NB_O, nbc, 32])
                offS_b = offS3h[:, :, bsl].unsqueeze(3) \
                    .broadcast_to([P, NB_O, nbc, 32])
                we = dq.tile([P, NB_O, bcc], F16, name="we")
                nc.vector.tensor_tensor(we, lo, S_b, Alu.mult)
                wo = dq.tile([P, NB_O, bcc], F16, name="wo")
                nc.vector.tensor_tensor(wo, hi, S_b, Alu.mult)
                w_nat = dq.tile([P, NB_O, 128 * c], F16, name="w_nat", bufs=4)
                nc.vector.tensor_tensor(
                    w_nat[:, :, 0::2], we, offS_b, Alu.subtract)
                nc.vector.tensor_tensor(
                    w_nat[:, :, 1::2], wo, offS_b, Alu.subtract)
                chain.append(nc.sync.dma_start(
                    wds[ci].rearrange("(a p) c -> p a c", p=P), w_nat))
                chain.append(nc.scalar.dma_start(out=wts[ci],
                                                 in_=wds[ci][:, :],
                                                 transpose=True))

            # ---- third x block (256 tokens)
            t = xt_pool.tile([P, KT, X_BLOCKS[2]], F16, name="xtr2", bufs=1)
            xi = nc.scalar.dma_start(
                out=t, in_=x[r0:r0 + X_BLOCKS[2], :], transpose=True)
            xinsts.append(xi)
            chain.append(xi)
            xtiles.append(t)
            r0 += X_BLOCKS[2]

            def evac(ps, row0):
                ob = ob_pool.tile([P, O_C], F16, name="ob")
                nc.vector.tensor_copy(ob, ps)
                nc.sync.dma_start(out[row0:row0 + P, :], ob)

            # wave A: blocks 0,1 interleaved, kk-outer (2 MMs per arriving
            # k-tile keeps the PE fed while W chunks trickle in).
            psA = [ps_pool.tile([P, O_C], F32, name="ps") for _ in range(2)]
            for kk in range(KT):
                ci, j = kk2chunk[kk]
                for b in (0, 1):
                    nc.tensor.matmul(
                        psA[b], xtiles[b][:, kk, 0:P], wts[ci][:, j, :],
                        start=(kk == 0), stop=(kk == KT - 1))
            # wave B: block 2, kk-outer
            psB = [ps_pool.tile([P, O_C], F32, name="ps") for _ in range(2)]
            for kk in range(KT):
                ci, j = kk2chunk[kk]
                for st in range(2):
                    nc.tensor.matmul(
                        psB[st], xtiles[2][:, kk, st * P:(st + 1) * P],
                        wts[ci][:, j, :],
                        start=(kk == 0), stop=(kk == KT - 1))
            for b in (0, 1):
                evac(psA[b], b * P)
            for st in range(2):
                evac(psB[st], 256 + st * P)

            # steady blocks: st-outer, kk-inner (W fully resident)
            row0 = 512
            for bi in range(N_RAMP, len(X_BLOCKS)):
                tb = X_BLOCKS[bi]
                xt = xt_pool.tile([P, KT, tb], F16, name="xt")
                xi = nc.scalar.dma_start(
                    out=xt, in_=x[r0:r0 + tb, :], transpose=True)
                xinsts.append(xi)
                chain.append(xi)
                r0 += tb
                for st in range(tb // P):
                    ps = ps_pool.tile([P, O_C], F32, name="ps")
                    for kk in range(KT):
                        ci, j = kk2chunk[kk]
                        nc.tensor.matmul(
                            ps, xt[:, kk, st * P:(st + 1) * P],
                            wts[ci][:, j, :],
                            start=(kk == 0), stop=(kk == KT - 1))
                    evac(ps, row0)
                    row0 += P

            for a, b in zip(chain[1:], chain):
                tile_rust_add_dep(a.ins, b.ins, True, "global dma order")

    nc.compile()
    return nc


_NC_CACHE = {}


def _get_nc(tokens=TOKENS):
    if tokens not in _NC_CACHE:
        _NC_CACHE[tokens] = _build(tokens)
    return _NC_CACHE[tokens]


def _shard(inputs):
    x = np.ascontiguousarray(np.asarray(inputs["x"], dtype=np.float16))
    qw = np.asarray(inputs["quantized_weight"], dtype=np.int32).astype(np.uint8)
    qam = np.asarray(inputs["quant_absmax"], dtype=np.int32).astype(np.float32)
    qcode = np.asarray(inputs["quant_code"], dtype=np.float32)
    qoff = np.asarray(inputs["quant_offset"], dtype=np.float32)
    am2 = np.asarray(inputs["state2_absmax"], dtype=np.float32)
    c2 = np.asarray(inputs["state2_code"], dtype=np.float32)

    pb = O_C * BPR        # packed bytes per core
    nb1 = O_C * 64        # primary blocks per core
    nb2 = O_C * 16        # secondary blocks per core
    in_maps = []
    for c in range(N_CORES):
        scp = np.concatenate([
            qam[c * nb1:(c + 1) * nb1].reshape(O_C, 64),
            qcode[c * nb1:(c + 1) * nb1].reshape(O_C, 64),
            qoff[c * nb1:(c + 1) * nb1].reshape(O_C, 64),
            am2[c * nb2:(c + 1) * nb2].reshape(O_C, 16),
            c2[c * nb2:(c + 1) * nb2].reshape(O_C, 16),
        ], axis=1)
        in_maps.append({
            "x": x,
            "qw": np.ascontiguousarray(
                qw[c * pb:(c + 1) * pb].reshape(O_C, BPR)),
            "scp": np.ascontiguousarray(scp),
        })
    return in_maps


def _run(inputs, trace=False, trace_cores=None):
    nc = _get_nc()
    in_maps = _shard(inputs)
    res = run_bass_kernel_spmd(
        nc, in_maps, list(range(N_CORES)), trace=trace,
        trace_cores=trace_cores)
    out = np.concatenate([r["out"] for r in res.results], axis=1)
    return out, res


def kernel(**inputs) -> np.ndarray:
    out, _ = _run(inputs, trace=False)
    return out


# revision 11
# speedup vs baseline: 1.0697x; 1.0166x over previous
"""NF4-style 4-bit quantized linear: out = x @ dequant(w).T on 8 TRN2 NeuronCores.

Column-parallel sharding: core c owns output features [c*512, (c+1)*512) and the
matching slices of the packed weight + quant state arrays; x is replicated.

Per core:
  1. dequantize the 512x4096 weight slice on-chip (DVE, u8/f16 ops) in
     progressive k-chunks, round-tripping each chunk through DRAM with an xbar
     transpose to [k-partition, outf] layout,
  2. stream x through xbar transpose DMAs ([token, k] -> [k, token]) with two
     small 128-token lead blocks, and run the fp16 matmul on the PE array,
     accumulating in PSUM over 32 k-tiles.

Queue discipline (critical for the startup ramp):
  - ALL xbar transposes ride the ACT (scalar) HWDGE ring: concurrent
    transposes on the two HWDGE rings corrupt data (shared xbar), so they
    must be on one ring. Emission order: xtr0, xtr1, all W chunks, rest.
  - ALL plain DMAs (packed-scale load, packed-weight load, w-chunk stores,
    output writes) ride the SP (sync) HWDGE ring; plain HWDGE DMAs overlap
    in-flight transposes, unlike SWDGE (gpsimd) DMAs which Tile serializes
    against them. No gpsimd DMAs anywhere.
Host packs the five quant-state arrays into one f32 tensor (one DMA) and
provides qw as uint8; host gathers per-core outputs by concat along axis 1.
"""
import numpy as np

import concourse.bass as bass
import concourse.mybir as mybir
import concourse.tile as tile
from concourse import bacc
from concourse.tile_rust import add_dep_helper as tile_rust_add_dep
from concourse.bass_utils import run_bass_kernel_spmd

F16 = mybir.dt.float16
F32 = mybir.dt.float32
U8 = mybir.dt.uint8
Alu = mybir.AluOpType

P = 128
TOKENS = 8192
IN_F = 4096
OUT_F = 4096
N_CORES = 8
O_C = OUT_F // N_CORES          # 512 out features per core
KT = IN_F // P                  # 32 k-tiles
BPR = IN_F // 2                 # 2048 packed bytes per weight row
NB_O = O_C // P                 # 4 o-tiles of 128 rows

CHUNKS = [2, 2, 4, 8, 8, 8]    # k-tiles per dequant chunk (progressive)
KOFF = [0, 2, 4, 8, 16, 24]    # k-tile offset of each chunk
X_BLOCKS = [256, 256] + [512] * 15   # token blocks
N_RAMP = 2


def _build(tokens=TOKENS):
    nc = bacc.Bacc("TRN2", target_bir_lowering=False, debug=False,
                   enable_asserts=False)

    x = nc.dram_tensor("x", [tokens, IN_F], F16, kind="ExternalInput").ap()
    qw = nc.dram_tensor("qw", [O_C, BPR], U8, kind="ExternalInput").ap()
    # packed quant state: [qam | qcode | qoff | am2 | c2] along columns
    scp = nc.dram_tensor("scp", [O_C, 224], F32, kind="ExternalInput").ap()
    out = nc.dram_tensor("out", [tokens, O_C], F16, kind="ExternalOutput").ap()

    kk2chunk = []
    for ci, c in enumerate(CHUNKS):
        for j in range(c):
            kk2chunk.append((ci, j))

    with tile.TileContext(nc) as tc:
        with tc.tile_pool(name="wt_pool", bufs=1) as wt_pool, \
             tc.tile_pool(name="wdram", bufs=1, space="DRAM") as wdram, \
             tc.tile_pool(name="sc_pool", bufs=1) as sc_pool, \
             tc.tile_pool(name="dq", bufs=1) as dq, \
             tc.tile_pool(name="xt_pool", bufs=2) as xt_pool, \
             tc.tile_pool(name="ps_pool", bufs=8, space="PSUM") as ps_pool, \
             tc.tile_pool(name="ob_pool", bufs=2) as ob_pool:
            wts = [wt_pool.tile([P, c, O_C], F16, name=f"wt{ci}")
                   for ci, c in enumerate(CHUNKS)]
            wds = [wdram.tile([O_C, c * P], F16, name=f"wd{ci}")
                   for ci, c in enumerate(CHUNKS)]

            # Effectively all DMA traffic serializes against in-flight xbar
            # transposes, so pin ONE explicit global order tuned for the ramp:
            # loads, xtr0, xtr1, (store_i, wt_i)*, xtr2, steady xts.
            chain = []

            # ---- plain loads first (SP ring), then the two ramp x blocks
            qt_all = dq.tile([P, NB_O, BPR], U8, name="qt_all")
            chain.append(nc.sync.dma_start(
                qt_all, qw.rearrange("(a p) c -> p a c", p=P)))
            sc3 = sc_pool.tile([P, NB_O, 224], F32, name="sc3")
            chain.append(nc.sync.dma_start(
                sc3, scp.rearrange("(a p) c -> p a c", p=P)))

            xtiles, xinsts = [], []
            r0 = 0
            for bi in range(2):
                t = xt_pool.tile([P, KT, X_BLOCKS[bi]], F16,
                                 name=f"xtr{bi}", bufs=1)
                xi = nc.scalar.dma_start(out=t, in_=x[r0:r0 + X_BLOCKS[bi], :],
                                         transpose=True)
                xinsts.append(xi)
                chain.append(xi)
                xtiles.append(t)
                r0 += X_BLOCKS[bi]

            # ---- scale math on DVE -> f16 scale/offset tiles
            am3 = sc3[:, :, 0:64]
            cd3 = sc3[:, :, 64:128]
            of3 = sc3[:, :, 128:192]
            am23 = sc3[:, :, 192:208]
            c23 = sc3[:, :, 208:224]
            rc = sc_pool.tile([P, NB_O, 64], F32, name="rc")
            nc.vector.reciprocal(rc, cd3)
            s1 = sc_pool.tile([P, NB_O, 64], F32, name="s1")
            nc.vector.tensor_tensor(s1, am3, rc, Alu.mult)
            rc2 = sc_pool.tile([P, NB_O, 16], F32, name="rc2")
            nc.vector.reciprocal(rc2, c23)
            s2 = sc_pool.tile([P, NB_O, 16], F32, name="s2")
            nc.vector.tensor_tensor(s2, am23, rc2, Alu.mult)
            S3 = sc_pool.tile([P, NB_O, 64], F32, name="S3")
            nc.vector.tensor_tensor(
                S3, s1, s2.unsqueeze(3).broadcast_to([P, NB_O, 16, 4]), Alu.mult)
            offS3 = sc_pool.tile([P, NB_O, 64], F32, name="offS3")
            nc.vector.tensor_tensor(offS3, of3, S3, Alu.mult)
            S3h = sc_pool.tile([P, NB_O, 64], F16, name="S3h")
            nc.vector.tensor_copy(S3h, S3)
            offS3h = sc_pool.tile([P, NB_O, 64], F16, name="offS3h")
            nc.vector.tensor_copy(offS3h, offS3)

            # ---- dequant chunks: DVE -> store (SP) -> transpose (ACT)
            for ci, c in enumerate(CHUNKS):
                bcc = 64 * c
                nbc = 2 * c
                b0 = KOFF[ci] * 64
                qt = qt_all[:, :, b0:b0 + bcc]
                hi = dq.tile([P, NB_O, bcc], U8, name="hi")
                nc.vector.tensor_scalar(hi, qt, 4, None,
                                        Alu.logical_shift_right)
                lo = dq.tile([P, NB_O, bcc], U8, name="lo")
                nc.vector.tensor_scalar(lo, qt, 15, None, Alu.bitwise_and)
                bsl = slice(KOFF[ci] * 2, KOFF[ci] * 2 + nbc)
                S_b = S3h[:, :, bsl].unsqueeze(3) \
                    .broadcast_to([P, NB_O, nbc, 32])
                offS_b = offS3h[:, :, bsl].unsqueeze(3) \
                    .broadcast_to([P, NB_O, nbc, 32])
                we = dq.tile([P, NB_O, bcc], F16, name="we")
                nc.vector.tensor_tensor(we, lo, S_b, Alu.mult)
                wo = dq.tile([P, NB_O, bcc], F16, name="wo")
                nc.vector.tensor_tensor(wo, hi, S_b, Alu.mult)
                w_nat = dq.tile([P, NB_O, 128 * c], F16, name="w_nat", bufs=4)
                nc.vector.tensor_tensor(
                    w_nat[:, :, 0::2], we, offS_b, Alu.subtract)
                nc.vector.tensor_tensor(
                    w_nat[:, :, 1::2], wo, offS_b, Alu.subtract)
                chain.append(nc.sync.dma_start(
                    wds[ci].rearrange("(a p) c -> p a c", p=P), w_nat))
                chain.append(nc.scalar.dma_start(out=wts[ci],
                                                 in_=wds[ci][:, :],
                                                 transpose=True))

            def evac(ps, row0):
                ob = ob_pool.tile([P, O_C], F16, name="ob")
                nc.vector.tensor_copy(ob, ps)
                nc.sync.dma_start(out[row0:row0 + P, :], ob)

            # wave A: blocks 0,1 (256 tokens each) interleaved kk-outer --
            # 4 MMs per arriving k-tile absorb the W-chunk trickle cadence.
            psA = [ps_pool.tile([P, O_C], F32, name="ps") for _ in range(4)]
            for kk in range(KT):
                ci, j = kk2chunk[kk]
                for b in (0, 1):
                    for st in range(2):
                        nc.tensor.matmul(
                            psA[2 * b + st],
                            xtiles[b][:, kk, st * P:(st + 1) * P],
                            wts[ci][:, j, :],
                            start=(kk == 0), stop=(kk == KT - 1))
            for i in range(4):
                evac(psA[i], i * P)

            # steady blocks: st-outer, kk-inner (W fully resident)
            row0 = 512
            for bi in range(N_RAMP, len(X_BLOCKS)):
                tb = X_BLOCKS[bi]
                xt = xt_pool.tile([P, KT, tb], F16, name="xt")
                xi = nc.scalar.dma_start(
                    out=xt, in_=x[r0:r0 + tb, :], transpose=True)
                xinsts.append(xi)
                chain.append(xi)
                r0 += tb
                for st in range(tb // P):
                    ps = ps_pool.tile([P, O_C], F32, name="ps")
                    for kk in range(KT):
                        ci, j = kk2chunk[kk]
                        nc.tensor.matmul(
                            ps, xt[:, kk, st * P:(st + 1) * P],
                            wts[ci][:, j, :],
                            start=(kk == 0), stop=(kk == KT - 1))
                    evac(ps, row0)
                    row0 += P

            for a, b in zip(chain[1:], chain):
                tile_rust_add_dep(a.ins, b.ins, True, "global dma order")

    nc.compile()
    return nc


_NC_CACHE = {}


def _get_nc(tokens=TOKENS):
    if tokens not in _NC_CACHE:
        _NC_CACHE[tokens] = _build(tokens)
    return _NC_CACHE[tokens]


def _shard(inputs):
    x = np.ascontiguousarray(np.asarray(inputs["x"], dtype=np.float16))
    qw = np.asarray(inputs["quantized_weight"], dtype=np.int32).astype(np.uint8)
    qam = np.asarray(inputs["quant_absmax"], dtype=np.int32).astype(np.float32)
    qcode = np.asarray(inputs["quant_code"], dtype=np.float32)
    qoff = np.asarray(inputs["quant_offset"], dtype=np.float32)
    am2 = np.asarray(inputs["state2_absmax"], dtype=np.float32)
    c2 = np.asarray(inputs["state2_code"], dtype=np.float32)

    pb = O_C * BPR        # packed bytes per core
    nb1 = O_C * 64        # primary blocks per core
    nb2 = O_C * 16        # secondary blocks per core
    in_maps = []
    for c in range(N_CORES):
        scp = np.concatenate([
            qam[c * nb1:(c + 1) * nb1].reshape(O_C, 64),
            qcode[c * nb1:(c + 1) * nb1].reshape(O_C, 64),
            qoff[c * nb1:(c + 1) * nb1].reshape(O_C, 64),
            am2[c * nb2:(c + 1) * nb2].reshape(O_C, 16),
            c2[c * nb2:(c + 1) * nb2].reshape(O_C, 16),
        ], axis=1)
        in_maps.append({
            "x": x,
            "qw": np.ascontiguousarray(
                qw[c * pb:(c + 1) * pb].reshape(O_C, BPR)),
            "scp": np.ascontiguousarray(scp),
        })
    return in_maps


def _run(inputs, trace=False, trace_cores=None):
    nc = _get_nc()
    in_maps = _shard(inputs)
    res = run_bass_kernel_spmd(
        nc, in_maps, list(range(N_CORES)), trace=trace,
        trace_cores=trace_cores)
    out = np.concatenate([r["out"] for r in res.results], axis=1)
    return out, res


def kernel(**inputs) -> np.ndarray:
    out, _ = _run(inputs, trace=False)
    return out
